# revision 16
# baseline (speedup 1.0000x reference)
"""GCN forward (2x graph-conv + global max-pool + linear) on 8 TRN2 NeuronCores.

Reference computation (N=16384 nodes, 256 feats, 64 hid):
    h1 = relu(adj @ (x @ W1) + b1)          [N, 64]
    h2 = adj @ (h1 @ W2) + b2               [N, 2]
    out = max(h2, axis=0) @ W3.T + b3       [1, 1, 1]

Distribution (collective-free, one continuous DMA stream per core):
  pass A (row shard): core c computes h1 for its rows [c*2048, (c+1)*2048):
          psA = Delta.T @ adjT_fp8 + mt.T @ rsum   (4 i-strips in 2 psum
          banks, one k-sweep; Delta = fp8(2^sx*(x@W1 - m)) from HOST, 1 MiB)
  stage 3: delta_g = h1 @ W2 - c  (fp32), quantized fp8 on device
  pass B (column shard): partial h2 for ALL nodes from LOCAL delta_g only:
          psB[t, i] += delta_g_local.T @ adj_fp8[k_local, i]
          4 i-strips column-packed per psum bank; partial [2, 16384] per
          core DMA'd out; host sums the 8 partials.
Host: sum partials, + rsum*c + b2, max over nodes, @ W3.T + b3.

Key structure: every adj tile is [128, 4*4*512] = 1 MiB holding 4 k-chunks
x 4 i-strips x 512 (strip-interleaved so the PE never waits on a later
strip's DMA).  One host tensor T[cg, ig, kbg, p, kl, s, ii] serves both
passes: pass A of core c streams T[g, c] (g != c), pass B streams T[c, ig]
(ig != c), and the diagonal block T[c, c] (4 MiB) is loaded ONCE and reused
by both passes from SBUF => 60 MiB of adj DMA per core instead of 64.
Adjacency is fp8e4m3 (x2^sa so max < 240); fp8 noise is harmless because
the row-mean component rides on exact f32 sidecars (host rsum, mt, c).
Loads on the sync HWDGE queue are the pure adj stream; consts/delta and
the compute-dependent output stores ride the scalar HWDGE queue so they
never head-of-line-block the stream.
"""

import os
import sys

sys.path.insert(0, "/opt/trn_rl_repo")

import numpy as np
import ml_dtypes


def _install_ntff_hook_shim():
    """The image's `antenv` lacks `axon_hooks`, which bass_utils imports for
    trace=True under axon. Provide it, wired to the PJRT .so's NRT-profile
    C ABI (same thing trn_boot would have registered)."""
    import types
    if "antenv.axon_hooks" in sys.modules:
        return
    try:
        import antenv  # noqa: F401
        from trn_agent_boot.trn_boot import _ntff_profile_via_ctypes
        mod = types.ModuleType("antenv.axon_hooks")
        _state = {"hook": _ntff_profile_via_ctypes("/opt/axon/libaxon_pjrt.so")}
        mod.set_axon_ntff_profile_hook = lambda h: _state.update(hook=h)
        mod.get_axon_ntff_profile_hook = lambda: _state["hook"]
        sys.modules["antenv.axon_hooks"] = mod
    except Exception:
        pass


_install_ntff_hook_shim()

import concourse.bass as bass
import concourse.mybir as mybir
import concourse.tile as tile
from concourse import bacc
from concourse.bass_utils import run_bass_kernel_spmd

FP8_NP = ml_dtypes.float8_e4m3

P = 128          # partition dim
N_CORES = 8
N_NODES = 16384
N_FEAT = 256
N_HID = 64


class Cfg:
    def __init__(self, n=N_NODES, n_feat=N_FEAT, n_hid=N_HID, n_cores=N_CORES,
                 iw=512, adj_bufs=18, sa=21, sd=10, sx=4):
        self.n, self.n_feat, self.n_hid, self.n_cores = n, n_feat, n_hid, n_cores
        self.rows = n // n_cores       # rows per core (pass-A output rows,
                                       # == pass-B local contraction cols)
        self.iw = iw                   # i-strip width (psum free dim)
        self.ns = self.rows // iw      # strips per 2048-row block (= 4)
        self.klt = 4                   # k-chunks per tile
        self.kc = n // P               # total contraction chunks (128)
        self.nkt = self.kc // self.klt   # pass-A k-tiles (32)
        self.mcl = self.rows // P      # local node chunks (= pass-B k chunks)
        self.nbt = self.mcl // self.klt  # pass-B k-tiles per i-group (4)
        self.ngrp = n // self.rows     # 2048-blocks (= n_cores = 8)
        self.tw = self.klt * self.ns * iw  # tile free width (8192)
        self.adj_bufs = adj_bufs
        # fp8 scales (powers of 2, exact): adj x2^sa keeps max < 240;
        # Delta x2^sx on host; pass-B delta_g x2^sd on device.
        # psA holds 2^(sa+sx)*h1T', psB 2^(sa+sd)*partial_h2T.
        self.sa = sa
        self.sd = sd
        self.sx = sx
        assert self.rows == self.ns * iw and self.ngrp == n_cores


def build_nc(cfg: Cfg) -> bass.Bass:
    F32 = mybir.dt.float32
    FP8 = mybir.dt.float8e4
    n_hid, iw = cfg.n_hid, cfg.iw

    nc = bacc.Bacc("TRN2", target_bir_lowering=False)
    # Tile format (everywhere): t[p, (kl*ns + s)*iw + ii] = 2^sa *
    #   adj[row0 + s*iw + ii, col0 + (kl*P) + p] in fp8e4m3.
    # diag: the 4 tiles of the core's diagonal block (row0 = col0 = c*rows),
    # used by pass A (k-tiles 0..3) AND pass B (slot 0) from SBUF.
    diag_h = nc.declare_dram_parameter("diag", [cfg.klt, P, cfg.tw], FP8,
                                       isOutput=False)
    # adjA[j]: pass-A tiles, k-groups != c in ascending order (7 groups x 4)
    adjA_h = nc.declare_dram_parameter(
        "adjA", [(cfg.ngrp - 1) * cfg.klt, P, cfg.tw], FP8, isOutput=False)
    # adjB[j]: pass-B tiles, i-groups != c in ascending order (7 groups x 4)
    adjB_h = nc.declare_dram_parameter(
        "adjB", [(cfg.ngrp - 1) * cfg.klt, P, cfg.tw], FP8, isOutput=False)
    # delta[p, k*n_hid + h] = fp8(2^sx*(x@W1 - m))[perm-chunk k node 128k+p, h]
    delta_h = nc.declare_dram_parameter(
        "delta", [P, cfg.kc * n_hid], FP8, isOutput=False)
    # packed small consts: col 0 = b1 (dup halves), 1:3 = W2 (dup), 3:5 = c2,
    # row 0 cols 5:5+n_hid = mt
    cst_h = nc.declare_dram_parameter("cst", [P, 5 + n_hid], F32,
                                      isOutput=False)
    rs_h = nc.declare_dram_parameter("rsum", [1, cfg.rows], F32, isOutput=False)
    # out[slot][32*s + t, ii] = 2^(sa+sd) * partial_h2[ig(slot)*rows
    #   + s*iw + ii, t]   (other partition rows are zero)
    out_h = nc.declare_dram_parameter(
        "out", [cfg.ngrp, P, iw], F32, isOutput=True)

    with tile.TileContext(nc, num_cores=cfg.n_cores) as tc:
        with (
            tc.tile_pool(name="const", bufs=1) as const_pool,
            tc.tile_pool(name="dlt", bufs=1) as dlt_pool,
            tc.tile_pool(name="diagp", bufs=1) as diag_pool,
            tc.tile_pool(name="h1tp", bufs=1) as h1t_pool,
            tc.tile_pool(name="adjp", bufs=cfg.adj_bufs) as adj_pool,
            tc.tile_pool(name="gp", bufs=1) as g_pool,
            tc.tile_pool(name="op", bufs=2) as out_pool,
            tc.tile_pool(name="psAp", bufs=1, space="PSUM") as psA_pool,
            tc.tile_pool(name="ps3p", bufs=2, space="PSUM") as ps3_pool,
            tc.tile_pool(name="psBp", bufs=2, space="PSUM") as psB_pool,
        ):
            # ---- pinned diagonal tiles: FIRST on the sync queue
            diag_t = []
            for d in range(cfg.klt):
                t = diag_pool.tile([P, cfg.tw], FP8, tag=f"diag{d}",
                                   name=f"diag_sb{d}")
                nc.sync.dma_start(out=t[:, :], in_=diag_h[d])
                diag_t.append(t)
            # consts + delta on the scalar queue (parallel with the stream)
            cst_sb = const_pool.tile([P, 5 + n_hid], F32)
            nc.scalar.dma_start(out=cst_sb[:, :], in_=cst_h[:, :])
            rs_sb = const_pool.tile([1, cfg.rows], F32)
            nc.scalar.dma_start(out=rs_sb[:, :], in_=rs_h[:, :])
            dlt_sb = dlt_pool.tile([P, cfg.kc * n_hid], FP8)
            nc.scalar.dma_start(out=dlt_sb[:, :], in_=delta_h[:, :])
            b1_sb = cst_sb[:, 0:1]
            w2_sb = cst_sb[:, 1:3]
            c2_sb = cst_sb[:, 3:5]
            mt_sb = cst_sb[0:1, 5:5 + n_hid]

            # ---- pass A: one k-sweep; strip s -> psum bank s//2, rows
            # [(s%2)*64, +64), PE columns [(s%2)*64, +64)
            psA = [psA_pool.tile([P, iw], F32, tag=f"psA{b}", name=f"psA{b}")
                   for b in range(2)]
            for kt in range(cfg.nkt):
                if kt < cfg.klt:
                    at = diag_t[kt]
                else:
                    at = adj_pool.tile([P, cfg.tw], FP8, tag="at")
                    nc.sync.dma_start(out=at[:, :], in_=adjA_h[kt - cfg.klt])
                for kl in range(cfg.klt):
                    k = kt * cfg.klt + kl
                    for s in range(cfg.ns):
                        u = s % 2
                        nc.tensor.matmul(
                            psA[s // 2][u * n_hid:(u + 1) * n_hid, :],
                            lhsT=dlt_sb[:, k * n_hid:(k + 1) * n_hid],
                            rhs=at[:, (kl * cfg.ns + s) * iw:
                                   (kl * cfg.ns + s + 1) * iw],
                            start=(k == 0), stop=False,
                            tile_position=(0, u * n_hid),
                            skip_group_check=True,
                        )
            for s in range(cfg.ns):
                u = s % 2
                nc.tensor.matmul(
                    psA[s // 2][u * n_hid:(u + 1) * n_hid, :],
                    lhsT=mt_sb,
                    rhs=rs_sb[:, s * iw:(s + 1) * iw],
                    start=False, stop=True,
                    tile_position=(0, u * n_hid),
                    skip_group_check=True,
                )
            # ---- evacuate + stage 3: h1 = relu(2^-(sa+sx)*psA + b1);
            # delta_g = h1 @ W2 - c, node-major in gl
            h1t_sb = h1t_pool.tile([P, 2 * iw], F32)
            gl_sb = g_pool.tile([P, 2 * cfg.mcl], F32)
            for b in range(2):
                nc.scalar.activation(
                    h1t_sb[:, b * iw:(b + 1) * iw], psA[b][:, :],
                    mybir.ActivationFunctionType.Relu,
                    bias=b1_sb,
                    scale=float(2.0 ** -(cfg.sa + cfg.sx)),
                )
                for u in range(2):
                    for ml in range(iw // P):
                        m = (2 * b + u) * (iw // P) + ml
                        ps3 = ps3_pool.tile([P, 2], F32, tag="ps3")
                        nc.tensor.matmul(
                            ps3[:, :],
                            lhsT=h1t_sb[u * n_hid:(u + 1) * n_hid,
                                        b * iw + ml * P:b * iw + (ml + 1) * P],
                            rhs=w2_sb[u * n_hid:(u + 1) * n_hid, :],
                            start=True, stop=True,
                        )
                        nc.vector.tensor_sub(
                            gl_sb[:, 2 * m:2 * m + 2], ps3[:, :], c2_sb)
            # quantize delta_g: g8[p, 2k+t] = fp8(2^sd * delta_g[128k+p, t])
            g8_sb = g_pool.tile([P, 2 * cfg.mcl], FP8)
            nc.scalar.activation(
                g8_sb[:, :], gl_sb[:, :],
                mybir.ActivationFunctionType.Copy, scale=float(2 ** cfg.sd))

            # ---- pass B: slot 0 = the pinned diagonal (no DMA, fills the
            # transition); slots 1..7 stream T[c, ig] for ig != c.
            # strip s at psum rows/PE cols [32s, 32s+2).
            for slot in range(cfg.ngrp):
                psB = psB_pool.tile([P, iw], F32, tag="psB")
                # zero the junk rows so the full-bank evacuation below reads
                # initialized data (matmul start=True only clears has_written
                # bits; it does not zero untouched rows)
                nc.vector.memset(psB[:, :], 0.0)
                if slot == 0:
                    tiles = diag_t
                else:
                    tiles = []
                    for d in range(cfg.nbt):
                        at = adj_pool.tile([P, cfg.tw], FP8, tag="at")
                        nc.sync.dma_start(
                            out=at[:, :],
                            in_=adjB_h[(slot - 1) * cfg.nbt + d])
                        tiles.append(at)
                for d in range(cfg.nbt):
                    for kl in range(cfg.klt):
                        kb = d * cfg.klt + kl
                        for s in range(cfg.ns):
                            nc.tensor.matmul(
                                psB[32 * s:32 * s + 2, :],
                                lhsT=g8_sb[:, 2 * kb:2 * (kb + 1)],
                                rhs=tiles[d][:, (kl * cfg.ns + s) * iw:
                                             (kl * cfg.ns + s + 1) * iw],
                                start=(kb == 0), stop=(kb == cfg.mcl - 1),
                                tile_position=(0, 32 * s),
                                skip_group_check=True,
                            )
                ob_t = out_pool.tile([P, iw], F32, tag="ob")
                nc.vector.tensor_copy(ob_t[:, :], psB[:, :])
                nc.scalar.dma_start(out=out_h[slot], in_=ob_t[:, :])
    nc.compile()
    return nc


def shard_inputs(cfg: Cfg, x, adj, W1, b1, W2):
    """Host-side prep: quantize adj once, build the unified tile tensor T,
    compute Delta = fp8(2^sx*(x@W1 - m)) and the exactness sidecars."""
    x = np.asarray(x, dtype=np.float32)
    adj = np.asarray(adj, dtype=np.float32)
    W1f = np.asarray(W1, dtype=np.float32)
    b1f = np.asarray(b1, dtype=np.float32)
    W2f = np.asarray(W2, dtype=np.float32)
    n, nh, nc_, ns, klt = cfg.n, cfg.n_hid, cfg.n_cores, cfg.ns, cfg.klt

    # --- stage 1 on host (f32): Delta node-major, fp8
    sxf = np.float32(2.0 ** cfg.sx)
    xW1 = x @ W1f                                                # [n, 64]
    m = xW1.mean(axis=0, dtype=np.float64).astype(np.float32)
    Q = (xW1 - m) * sxf                                          # 2^sx-scaled
    Qq = Q.astype(FP8_NP)
    Qqf = Qq.astype(np.float32)
    assert np.isfinite(Qqf).all(), "Delta overflows fp8 range"
    eps = (Qqf - Q).mean(axis=0, dtype=np.float64).astype(np.float32)
    # pass-A correction lhsT (2^(sa+sx)-scaled psum units per unit rowsum):
    # adj@xW1 ~= adj@Qq/2^sx + rsum*(m - eps/2^sx)
    mt = (m * sxf - eps) * np.float32(2.0 ** cfg.sa)             # [64]

    # packed consts [P, 5+nh]: b1 | W2 | c2 | mt (row 0)
    idx = np.arange(0, n, max(1, n // 256))
    g_sub = np.maximum(adj[idx] @ xW1 + b1f, 0.0) @ W2f
    c_est = g_sub.mean(axis=0).astype(np.float32)                # [2]
    cst = np.zeros((P, 5 + nh), dtype=np.float32)
    cst[:, 0] = np.concatenate([b1f, b1f])
    cst[:, 1:3] = np.vstack([W2f, W2f])
    cst[:, 3:5] = c_est[None, :]
    cst[0, 5:5 + nh] = mt
    rsum = adj.sum(axis=1, dtype=np.float64).astype(np.float32)  # [n]

    # --- adjacency: quantize once (transposed), build the tile tensor
    #   T[cg, ig, kbg, p, kl, s, ii] = adj8[ig*rows + s*iw + ii,
    #                                       cg*rows + kbg*512 + kl*128 + p]
    saf = np.float32(2.0 ** cfg.sa)
    adjT = np.ascontiguousarray(adj.T)
    adj8T = (adjT * saf).astype(FP8_NP)                          # [col, row]
    del adjT
    T = np.ascontiguousarray(
        adj8T.reshape(nc_, klt, klt, P, nc_, ns, cfg.iw)
        .transpose(0, 4, 1, 3, 2, 5, 6)).reshape(
        nc_, nc_, klt, P, cfg.tw)
    del adj8T

    # delta chunks in per-core permuted order: group c first, rest ascending
    Qq4 = Qq.reshape(nc_, cfg.mcl, P, nh)       # (group, chunk-in-group, p, h)

    in_maps = []
    order_igs = []
    for c in range(nc_):
        others = [g for g in range(nc_) if g != c]
        diag = np.ascontiguousarray(T[c, c])
        adjA = np.ascontiguousarray(T[others, c]).reshape(-1, P, cfg.tw)
        adjB = np.ascontiguousarray(T[c, others]).reshape(-1, P, cfg.tw)
        dlt = np.ascontiguousarray(
            Qq4[[c] + others].reshape(cfg.kc, P, nh)
            .transpose(1, 0, 2)).reshape(P, cfg.kc * nh)
        rs = np.ascontiguousarray(
            rsum[c * cfg.rows:(c + 1) * cfg.rows].reshape(1, cfg.rows))
        in_maps.append({"diag": diag, "adjA": adjA, "adjB": adjB,
                        "delta": dlt, "cst": cst, "rsum": rs})
        order_igs.append([c] + others)
    return in_maps, rsum, c_est, order_igs


def finish_on_host(cfg: Cfg, per_core_out, rsum, c_est, b2, W3, b3):
    """per_core_out: [n_cores, nig(global order), P, iw]; rows {32s, 32s+1}
    hold the scaled partial h2 transpose.  Sum partials, add rsum*c + b2,
    max over nodes, @ W3.T + b3."""
    b2 = np.asarray(b2, dtype=np.float32)
    W3 = np.asarray(W3, dtype=np.float32)
    b3 = np.asarray(b3, dtype=np.float32)
    rows = np.array([32 * s + t for s in range(cfg.ns) for t in range(2)])
    tot = per_core_out.sum(axis=0, dtype=np.float64)   # [nig, P, iw]
    # [ig, (s, t), ii] -> h2[ig*rows + s*iw + ii, t]
    tot = tot[:, rows, :].reshape(cfg.ngrp, cfg.ns, 2, cfg.iw)
    tot = tot.transpose(0, 1, 3, 2)
    h2 = (tot.reshape(cfg.n, 2) * (2.0 ** -(cfg.sa + cfg.sd))
          + rsum.astype(np.float64)[:, None] * c_est[None, :] + b2[None, :])
    pooled = h2.max(axis=0).astype(np.float32)                     # [2]
    out = pooled[None, None, :] @ W3.T + b3                        # [1,1,1]
    return out.astype(np.float32)


_NC_CACHE: dict = {}
LAST_RESULT = None  # BassKernelResults of the most recent run (for test.py)


def kernel(x, adj, W1, b1, W2, b2, W3, b3):
    cfg = Cfg()
    x = np.asarray(x)
    assert x.shape == (cfg.n, cfg.n_feat), x.shape
    if "nc" not in _NC_CACHE:
        _NC_CACHE["nc"] = build_nc(cfg)
    nc = _NC_CACHE["nc"]

    in_maps, rsum, c_est, order_igs = shard_inputs(cfg, x, adj, W1, b1, W2)
    trace = os.environ.get("GCN_TRACE", "0") == "1"
    res = run_bass_kernel_spmd(
        nc, in_maps, core_ids=list(range(cfg.n_cores)), trace=trace)
    global LAST_RESULT
    LAST_RESULT = res
    per_core = []
    for c, r in enumerate(res.results):
        arr = np.asarray(r["out"], dtype=np.float32)   # [slots, P, iw]
        inv = np.argsort(order_igs[c])                 # slot -> global ig
        per_core.append(arr[inv])
    per_core = np.stack(per_core)
    return finish_on_host(cfg, per_core, rsum, c_est, b2, W3, b3)


# revision 21
# speedup vs baseline: 1.0035x; 1.0035x over previous
"""GCN forward (2x graph-conv + global max-pool + linear) on 8 TRN2 NeuronCores.

Reference computation (N=16384 nodes, 256 feats, 64 hid):
    h1 = relu(adj @ (x @ W1) + b1)          [N, 64]
    h2 = adj @ (h1 @ W2) + b2               [N, 2]
    out = max(h2, axis=0) @ W3.T + b3       [1, 1, 1]

Distribution (collective-free, one continuous DMA stream per core):
  pass A (row shard): core c computes h1 for its rows [c*2048, (c+1)*2048):
          psA = Delta.T @ adjT_fp8 + mt.T @ rsum   (4 i-strips in 2 psum
          banks, one k-sweep; Delta = fp8(2^sx*(x@W1 - m)) from HOST, 1 MiB)
  stage 3: delta_g = h1 @ W2 - c  (fp32), quantized fp8 on device
  pass B (column shard): partial h2 for ALL nodes from LOCAL delta_g only:
          psB[t, i] += delta_g_local.T @ adj_fp8[k_local, i]
          4 i-strips column-packed per psum bank; partial [2, 16384] per
          core DMA'd out; host sums the 8 partials.
Host: sum partials, + rsum*c + b2, max over nodes, @ W3.T + b3.

Key structure: every adj tile is [128, 4*4*512] = 1 MiB holding 4 k-chunks
x 4 i-strips x 512 (strip-interleaved so the PE never waits on a later
strip's DMA).  One host tensor T[cg, ig, kbg, p, kl, s, ii] serves both
passes: pass A of core c streams T[g, c] (g != c), pass B streams T[c, ig]
(ig != c), and the diagonal block T[c, c] (4 MiB) is loaded ONCE and reused
by both passes from SBUF => 60 MiB of adj DMA per core instead of 64.
Adjacency is fp8e4m3 (x2^sa so max < 240); fp8 noise is harmless because
the row-mean component rides on exact f32 sidecars (host rsum, mt, c).
Loads on the sync HWDGE queue are the pure adj stream; consts/delta and
the compute-dependent output stores ride the scalar HWDGE queue so they
never head-of-line-block the stream.
"""

import os
import sys

sys.path.insert(0, "/opt/trn_rl_repo")

import numpy as np
import ml_dtypes


def _install_ntff_hook_shim():
    """The image's `antenv` lacks `axon_hooks`, which bass_utils imports for
    trace=True under axon. Provide it, wired to the PJRT .so's NRT-profile
    C ABI (same thing trn_boot would have registered)."""
    import types
    if "antenv.axon_hooks" in sys.modules:
        return
    try:
        import antenv  # noqa: F401
        from trn_agent_boot.trn_boot import _ntff_profile_via_ctypes
        mod = types.ModuleType("antenv.axon_hooks")
        _state = {"hook": _ntff_profile_via_ctypes("/opt/axon/libaxon_pjrt.so")}
        mod.set_axon_ntff_profile_hook = lambda h: _state.update(hook=h)
        mod.get_axon_ntff_profile_hook = lambda: _state["hook"]
        sys.modules["antenv.axon_hooks"] = mod
    except Exception:
        pass


_install_ntff_hook_shim()

import concourse.bass as bass
import concourse.mybir as mybir
import concourse.tile as tile
from concourse import bacc
from concourse.bass_utils import run_bass_kernel_spmd

FP8_NP = ml_dtypes.float8_e4m3

P = 128          # partition dim
N_CORES = 8
N_NODES = 16384
N_FEAT = 256
N_HID = 64


class Cfg:
    def __init__(self, n=N_NODES, n_feat=N_FEAT, n_hid=N_HID, n_cores=N_CORES,
                 iw=512, adj_bufs=4, sa=21, sd=10, sx=4):
        self.n, self.n_feat, self.n_hid, self.n_cores = n, n_feat, n_hid, n_cores
        self.rows = n // n_cores       # rows per core (pass-A output rows,
                                       # == pass-B local contraction cols)
        self.iw = iw                   # i-strip width (psum free dim)
        self.ns = self.rows // iw      # strips per 2048-row block (= 4)
        self.klt = 4                   # k-chunks per tile
        self.kc = n // P               # total contraction chunks (128)
        self.nkt = self.kc // self.klt   # pass-A k-tiles (32)
        self.mcl = self.rows // P      # local node chunks (= pass-B k chunks)
        self.nbt = self.mcl // self.klt  # pass-B k-tiles per i-group (4)
        self.ngrp = n // self.rows     # 2048-blocks (= n_cores = 8)
        self.tw = self.klt * self.ns * iw  # tile free width (8192)
        self.adj_bufs = adj_bufs
        # fp8 scales (powers of 2, exact): adj x2^sa keeps max < 240;
        # Delta x2^sx on host; pass-B delta_g x2^sd on device.
        # psA holds 2^(sa+sx)*h1T', psB 2^(sa+sd)*partial_h2T.
        self.sa = sa
        self.sd = sd
        self.sx = sx
        assert self.rows == self.ns * iw and self.ngrp == n_cores


def build_nc(cfg: Cfg) -> bass.Bass:
    F32 = mybir.dt.float32
    FP8 = mybir.dt.float8e4
    n_hid, iw = cfg.n_hid, cfg.iw

    nc = bacc.Bacc("TRN2", target_bir_lowering=False)
    # Tile format (everywhere): t[p, (kl*ns + s)*iw + ii] = 2^sa *
    #   adj[row0 + s*iw + ii, col0 + (kl*P) + p] in fp8e4m3.
    # diag: the 4 tiles of the core's diagonal block (row0 = col0 = c*rows),
    # used by pass A (k-tiles 0..3) AND pass B (slot 0) from SBUF.
    diag_h = nc.declare_dram_parameter("diag", [cfg.klt, P, cfg.tw], FP8,
                                       isOutput=False)
    # adjA[g]: pass-A 4-MiB groups (4 sub-tiles concatenated on the free
    # dim), k-groups != c in ascending order
    adjA_h = nc.declare_dram_parameter(
        "adjA", [cfg.ngrp - 1, P, cfg.klt * cfg.tw], FP8, isOutput=False)
    # adjB[g]: pass-B 4-MiB groups, i-groups != c in ascending order
    adjB_h = nc.declare_dram_parameter(
        "adjB", [cfg.ngrp - 1, P, cfg.klt * cfg.tw], FP8, isOutput=False)
    # delta[p, k*n_hid + h] = fp8(2^sx*(x@W1 - m))[perm-chunk k node 128k+p, h]
    delta_h = nc.declare_dram_parameter(
        "delta", [P, cfg.kc * n_hid], FP8, isOutput=False)
    # packed small consts: col 0 = b1 (dup halves), 1:3 = W2 (dup), 3:5 = c2,
    # row 0 cols 5:5+n_hid = mt
    cst_h = nc.declare_dram_parameter("cst", [P, 5 + n_hid], F32,
                                      isOutput=False)
    rs_h = nc.declare_dram_parameter("rsum", [1, cfg.rows], F32, isOutput=False)
    # out[slot][32*s + t, ii] = 2^(sa+sd) * partial_h2[ig(slot)*rows
    #   + s*iw + ii, t]   (other partition rows are zero)
    out_h = nc.declare_dram_parameter(
        "out", [cfg.ngrp, P, iw], F32, isOutput=True)

    with tile.TileContext(nc, num_cores=cfg.n_cores) as tc:
        with (
            tc.tile_pool(name="const", bufs=1) as const_pool,
            tc.tile_pool(name="dlt", bufs=1) as dlt_pool,
            tc.tile_pool(name="diagp", bufs=1) as diag_pool,
            tc.tile_pool(name="h1tp", bufs=1) as h1t_pool,
            tc.tile_pool(name="adjp", bufs=cfg.adj_bufs) as adj_pool,
            tc.tile_pool(name="gp", bufs=1) as g_pool,
            tc.tile_pool(name="op", bufs=2) as out_pool,
            tc.tile_pool(name="psAp", bufs=1, space="PSUM") as psA_pool,
            tc.tile_pool(name="ps3p", bufs=2, space="PSUM") as ps3_pool,
            tc.tile_pool(name="psBp", bufs=2, space="PSUM") as psB_pool,
        ):
            # ---- pinned diagonal tiles: FIRST on the sync queue
            diag_t = []
            for d in range(cfg.klt):
                t = diag_pool.tile([P, cfg.tw], FP8, tag=f"diag{d}",
                                   name=f"diag_sb{d}")
                nc.sync.dma_start(out=t[:, :], in_=diag_h[d])
                diag_t.append(t)
            # consts + delta on the scalar queue (parallel with the stream)
            cst_sb = const_pool.tile([P, 5 + n_hid], F32)
            nc.scalar.dma_start(out=cst_sb[:, :], in_=cst_h[:, :])
            rs_sb = const_pool.tile([1, cfg.rows], F32)
            nc.scalar.dma_start(out=rs_sb[:, :], in_=rs_h[:, :])
            dlt_sb = dlt_pool.tile([P, cfg.kc * n_hid], FP8)
            nc.scalar.dma_start(out=dlt_sb[:, :], in_=delta_h[:, :])
            b1_sb = cst_sb[:, 0:1]
            w2_sb = cst_sb[:, 1:3]
            c2_sb = cst_sb[:, 3:5]
            mt_sb = cst_sb[0:1, 5:5 + n_hid]

            # ---- pass A: one k-sweep; strip s -> psum bank s//2, rows
            # [(s%2)*64, +64), PE columns [(s%2)*64, +64).
            # k-tiles 0..3 come from the pinned diagonal; the rest stream as
            # 4-MiB groups, alternating between the two HWDGE rings so the
            # per-DMA completion latency never paces the stream.
            psA = [psA_pool.tile([P, iw], F32, tag=f"psA{b}", name=f"psA{b}")
                   for b in range(2)]
            for gt in range(cfg.ngrp):
                if gt == 0:
                    tiles = [(diag_t[d], 0) for d in range(cfg.klt)]
                else:
                    eng = nc.sync if gt % 2 == 1 else nc.scalar
                    big = adj_pool.tile([P, cfg.klt * cfg.tw], FP8, tag="at")
                    eng.dma_start(out=big[:, :], in_=adjA_h[gt - 1])
                    tiles = [(big, d * cfg.tw) for d in range(cfg.klt)]
                for d in range(cfg.klt):
                    at, off = tiles[d]
                    kt = gt * cfg.klt + d
                    for kl in range(cfg.klt):
                        k = kt * cfg.klt + kl
                        for s in range(cfg.ns):
                            u = s % 2
                            nc.tensor.matmul(
                                psA[s // 2][u * n_hid:(u + 1) * n_hid, :],
                                lhsT=dlt_sb[:, k * n_hid:(k + 1) * n_hid],
                                rhs=at[:, off + (kl * cfg.ns + s) * iw:
                                       off + (kl * cfg.ns + s + 1) * iw],
                                start=(k == 0), stop=False,
                                tile_position=(0, u * n_hid),
                                skip_group_check=True,
                            )
            for s in range(cfg.ns):
                u = s % 2
                nc.tensor.matmul(
                    psA[s // 2][u * n_hid:(u + 1) * n_hid, :],
                    lhsT=mt_sb,
                    rhs=rs_sb[:, s * iw:(s + 1) * iw],
                    start=False, stop=True,
                    tile_position=(0, u * n_hid),
                    skip_group_check=True,
                )
            # ---- evacuate + stage 3: h1 = relu(2^-(sa+sx)*psA + b1);
            # delta_g = h1 @ W2 - c, node-major in gl
            h1t_sb = h1t_pool.tile([P, 2 * iw], F32)
            gl_sb = g_pool.tile([P, 2 * cfg.mcl], F32)
            for b in range(2):
                nc.scalar.activation(
                    h1t_sb[:, b * iw:(b + 1) * iw], psA[b][:, :],
                    mybir.ActivationFunctionType.Relu,
                    bias=b1_sb,
                    scale=float(2.0 ** -(cfg.sa + cfg.sx)),
                )
                for u in range(2):
                    for ml in range(iw // P):
                        m = (2 * b + u) * (iw // P) + ml
                        ps3 = ps3_pool.tile([P, 2], F32, tag="ps3")
                        nc.tensor.matmul(
                            ps3[:, :],
                            lhsT=h1t_sb[u * n_hid:(u + 1) * n_hid,
                                        b * iw + ml * P:b * iw + (ml + 1) * P],
                            rhs=w2_sb[u * n_hid:(u + 1) * n_hid, :],
                            start=True, stop=True,
                        )
                        nc.vector.tensor_sub(
                            gl_sb[:, 2 * m:2 * m + 2], ps3[:, :], c2_sb)
            # quantize delta_g: g8[p, 2k+t] = fp8(2^sd * delta_g[128k+p, t])
            g8_sb = g_pool.tile([P, 2 * cfg.mcl], FP8)
            nc.scalar.activation(
                g8_sb[:, :], gl_sb[:, :],
                mybir.ActivationFunctionType.Copy, scale=float(2 ** cfg.sd))

            # ---- pass B: slot 0 = the pinned diagonal (no DMA, fills the
            # transition); slots 1..7 stream T[c, ig] for ig != c.
            # strip s at psum rows/PE cols [32s, 32s+2).
            for slot in range(cfg.ngrp):
                psB = psB_pool.tile([P, iw], F32, tag="psB")
                # zero the junk rows so the full-bank evacuation below reads
                # initialized data (matmul start=True only clears has_written
                # bits; it does not zero untouched rows)
                nc.vector.memset(psB[:, :], 0.0)
                if slot == 0:
                    tiles = [(diag_t[d], 0) for d in range(cfg.nbt)]
                else:
                    eng = nc.sync if slot % 2 == 1 else nc.scalar
                    big = adj_pool.tile([P, cfg.klt * cfg.tw], FP8, tag="at")
                    eng.dma_start(out=big[:, :], in_=adjB_h[slot - 1])
                    tiles = [(big, d * cfg.tw) for d in range(cfg.nbt)]
                for d in range(cfg.nbt):
                    at, off = tiles[d]
                    for kl in range(cfg.klt):
                        kb = d * cfg.klt + kl
                        for s in range(cfg.ns):
                            nc.tensor.matmul(
                                psB[32 * s:32 * s + 2, :],
                                lhsT=g8_sb[:, 2 * kb:2 * (kb + 1)],
                                rhs=at[:, off + (kl * cfg.ns + s) * iw:
                                       off + (kl * cfg.ns + s + 1) * iw],
                                start=(kb == 0), stop=(kb == cfg.mcl - 1),
                                tile_position=(0, 32 * s),
                                skip_group_check=True,
                            )
                ob_t = out_pool.tile([P, iw], F32, tag="ob")
                nc.vector.tensor_copy(ob_t[:, :], psB[:, :])
                nc.gpsimd.dma_start(out=out_h[slot], in_=ob_t[:, :])
    nc.compile()
    return nc


def shard_inputs(cfg: Cfg, x, adj, W1, b1, W2):
    """Host-side prep: quantize adj once, build the unified tile tensor T,
    compute Delta = fp8(2^sx*(x@W1 - m)) and the exactness sidecars."""
    x = np.asarray(x, dtype=np.float32)
    adj = np.asarray(adj, dtype=np.float32)
    W1f = np.asarray(W1, dtype=np.float32)
    b1f = np.asarray(b1, dtype=np.float32)
    W2f = np.asarray(W2, dtype=np.float32)
    n, nh, nc_, ns, klt = cfg.n, cfg.n_hid, cfg.n_cores, cfg.ns, cfg.klt

    # --- stage 1 on host (f32): Delta node-major, fp8
    sxf = np.float32(2.0 ** cfg.sx)
    xW1 = x @ W1f                                                # [n, 64]
    m = xW1.mean(axis=0, dtype=np.float64).astype(np.float32)
    Q = (xW1 - m) * sxf                                          # 2^sx-scaled
    Qq = Q.astype(FP8_NP)
    Qqf = Qq.astype(np.float32)
    assert np.isfinite(Qqf).all(), "Delta overflows fp8 range"
    eps = (Qqf - Q).mean(axis=0, dtype=np.float64).astype(np.float32)
    # pass-A correction lhsT (2^(sa+sx)-scaled psum units per unit rowsum):
    # adj@xW1 ~= adj@Qq/2^sx + rsum*(m - eps/2^sx)
    mt = (m * sxf - eps) * np.float32(2.0 ** cfg.sa)             # [64]

    # packed consts [P, 5+nh]: b1 | W2 | c2 | mt (row 0)
    idx = np.arange(0, n, max(1, n // 256))
    g_sub = np.maximum(adj[idx] @ xW1 + b1f, 0.0) @ W2f
    c_est = g_sub.mean(axis=0).astype(np.float32)                # [2]
    cst = np.zeros((P, 5 + nh), dtype=np.float32)
    cst[:, 0] = np.concatenate([b1f, b1f])
    cst[:, 1:3] = np.vstack([W2f, W2f])
    cst[:, 3:5] = c_est[None, :]
    cst[0, 5:5 + nh] = mt
    rsum = adj.sum(axis=1, dtype=np.float64).astype(np.float32)  # [n]

    # --- adjacency: quantize once (transposed), build the tile tensor
    #   T[cg, ig, kbg, p, kl, s, ii] = adj8[ig*rows + s*iw + ii,
    #                                       cg*rows + kbg*512 + kl*128 + p]
    saf = np.float32(2.0 ** cfg.sa)
    adjT = np.ascontiguousarray(adj.T)
    adj8T = (adjT * saf).astype(FP8_NP)                          # [col, row]
    del adjT
    T = np.ascontiguousarray(
        adj8T.reshape(nc_, klt, klt, P, nc_, ns, cfg.iw)
        .transpose(0, 4, 1, 3, 2, 5, 6)).reshape(
        nc_, nc_, klt, P, cfg.tw)
    del adj8T

    # delta chunks in per-core permuted order: group c first, rest ascending
    Qq4 = Qq.reshape(nc_, cfg.mcl, P, nh)       # (group, chunk-in-group, p, h)

    in_maps = []
    order_igs = []
    for c in range(nc_):
        others = [g for g in range(nc_) if g != c]
        diag = np.ascontiguousarray(T[c, c])
        # 4-MiB groups: the 4 sub-tiles of a block concatenated per partition
        adjA = np.ascontiguousarray(
            T[others, c].transpose(0, 2, 1, 3)).reshape(-1, P, klt * cfg.tw)
        adjB = np.ascontiguousarray(
            T[c, others].transpose(0, 2, 1, 3)).reshape(-1, P, klt * cfg.tw)
        dlt = np.ascontiguousarray(
            Qq4[[c] + others].reshape(cfg.kc, P, nh)
            .transpose(1, 0, 2)).reshape(P, cfg.kc * nh)
        rs = np.ascontiguousarray(
            rsum[c * cfg.rows:(c + 1) * cfg.rows].reshape(1, cfg.rows))
        in_maps.append({"diag": diag, "adjA": adjA, "adjB": adjB,
                        "delta": dlt, "cst": cst, "rsum": rs})
        order_igs.append([c] + others)
    return in_maps, rsum, c_est, order_igs


def finish_on_host(cfg: Cfg, per_core_out, rsum, c_est, b2, W3, b3):
    """per_core_out: [n_cores, nig(global order), P, iw]; rows {32s, 32s+1}
    hold the scaled partial h2 transpose.  Sum partials, add rsum*c + b2,
    max over nodes, @ W3.T + b3."""
    b2 = np.asarray(b2, dtype=np.float32)
    W3 = np.asarray(W3, dtype=np.float32)
    b3 = np.asarray(b3, dtype=np.float32)
    rows = np.array([32 * s + t for s in range(cfg.ns) for t in range(2)])
    tot = per_core_out.sum(axis=0, dtype=np.float64)   # [nig, P, iw]
    # [ig, (s, t), ii] -> h2[ig*rows + s*iw + ii, t]
    tot = tot[:, rows, :].reshape(cfg.ngrp, cfg.ns, 2, cfg.iw)
    tot = tot.transpose(0, 1, 3, 2)
    h2 = (tot.reshape(cfg.n, 2) * (2.0 ** -(cfg.sa + cfg.sd))
          + rsum.astype(np.float64)[:, None] * c_est[None, :] + b2[None, :])
    pooled = h2.max(axis=0).astype(np.float32)                     # [2]
    out = pooled[None, None, :] @ W3.T + b3                        # [1,1,1]
    return out.astype(np.float32)


_NC_CACHE: dict = {}
LAST_RESULT = None  # BassKernelResults of the most recent run (for test.py)


def kernel(x, adj, W1, b1, W2, b2, W3, b3):
    cfg = Cfg()
    x = np.asarray(x)
    assert x.shape == (cfg.n, cfg.n_feat), x.shape
    if "nc" not in _NC_CACHE:
        _NC_CACHE["nc"] = build_nc(cfg)
    nc = _NC_CACHE["nc"]

    in_maps, rsum, c_est, order_igs = shard_inputs(cfg, x, adj, W1, b1, W2)
    trace = os.environ.get("GCN_TRACE", "0") == "1"
    res = run_bass_kernel_spmd(
        nc, in_maps, core_ids=list(range(cfg.n_cores)), trace=trace)
    global LAST_RESULT
    LAST_RESULT = res
    per_core = []
    for c, r in enumerate(res.results):
        arr = np.asarray(r["out"], dtype=np.float32)   # [slots, P, iw]
        inv = np.argsort(order_igs[c])                 # slot -> global ig
        per_core.append(arr[inv])
    per_core = np.stack(per_core)
    return finish_on_host(cfg, per_core, rsum, c_est, b2, W3, b3)
